# revision 6
# baseline (speedup 1.0000x reference)
# FVSBN kernel for Trainium2: out = x @ (W * tril(-1)).T + b
#   x: [65536, 764] f32, W: [764, 764] f32, b: [764] f32 -> out: [65536, 764] f32
#
# Strategy: data-parallel over batch across 8 NeuronCores (8192 rows each).
# On each core we compute out^T = Wm^T-tiles.T @ x^T as a block-lower-triangular
# matmul: the strictly-lower-triangular mask means output tile row nt only needs
# contraction tiles dt <= nt (21 of 36 tile pairs).
#   - stationary operand (lhsT): Wm^T tile [128 d, 128 n]  (host packs the 21
#     used tiles contiguously)
#   - moving operand (rhs): x^T tile [128 d, 512 b]  (host pre-transposes x so
#     device DMAs are contiguous)
#   - psum [128 n, 512 b] accumulates over dt; eviction fuses the bias add
#     (per-partition scalar) on the vector engine.
# Host gathers by transposing each core's out^T back.

import numpy as np

B = 65536
D = 764
NCORES = 8
BPC = B // NCORES  # 8192 rows per core
P = 128
NT = 6  # ceil(764/128)
DP = NT * P  # 768, zero-padded depth
BB = 512  # matmul moving free dim == psum bank width (fp32)
PAIRS = [(nt, dt) for nt in range(NT) for dt in range(nt + 1)]
PAIR_IDX = {p: j for j, p in enumerate(PAIRS)}
NPAIR = len(PAIRS)  # 21

# device compute dtypes (np side); "float32r" = full-rate fp32 matmul mode
X_DT = "float16"  # dtype of x / W on device
OUT_DT = "float16"  # dtype out^T is written in
MM_DT = "float16"  # dtype the PE sees for the matmul operands


def _np_dt(name):
    import ml_dtypes

    return {
        "float32": np.float32,
        "float16": np.float16,
        "bfloat16": ml_dtypes.bfloat16,
    }[name]


def _build(bpc, x_dt_str=X_DT, out_dt_str=OUT_DT, mm_dt_str=MM_DT, reps=1):
    import concourse.mybir as mybir
    from concourse import bacc
    from concourse.tile import TileContext

    x_dt = getattr(mybir.dt, x_dt_str)
    out_dt = getattr(mybir.dt, out_dt_str)
    mm_dt = getattr(mybir.dt, mm_dt_str)
    f32 = mybir.dt.float32
    nbb = bpc // BB

    nc = bacc.Bacc("TRN2", target_bir_lowering=False, debug=False)
    xT = nc.dram_tensor("xt", [DP, bpc], x_dt, kind="ExternalInput")
    wt = nc.dram_tensor("wt", [P, NPAIR * P], x_dt, kind="ExternalInput")
    bias = nc.dram_tensor("bias", [P, NT], f32, kind="ExternalInput")
    outT = nc.dram_tensor("outt", [DP, bpc], out_dt, kind="ExternalOutput")

    def mm(ap):
        return ap if ap.dtype == mm_dt else ap.bitcast(mm_dt)

    with TileContext(nc) as tc:
        with (
            tc.tile_pool(name="wpool", bufs=1) as wpool,
            tc.tile_pool(name="bpool", bufs=1) as bpool,
            tc.tile_pool(name="xpool", bufs=3) as xpool,
            tc.tile_pool(name="opool", bufs=3) as opool,
            tc.tile_pool(name="pspool", bufs=8, space="PSUM") as pspool,
        ):
            w_sb = wpool.tile([P, NPAIR * P], x_dt)
            nc.sync.dma_start(out=w_sb, in_=wt.ap())
            bias_sb = bpool.tile([P, NT], f32)
            nc.sync.dma_start(out=bias_sb, in_=bias.ap())

            xv = xT.ap().rearrange("(t p) b -> p t b", p=P)
            ov = outT.ap().rearrange("(t p) b -> p t b", p=P)

            def body():
                for bg in range(nbb):
                    x_sb = xpool.tile([P, NT, BB], x_dt)
                    nc.sync.dma_start(out=x_sb, in_=xv[:, :, bg * BB : (bg + 1) * BB])
                    o_sb = opool.tile([P, NT, BB], out_dt)
                    for nt in range(NT):
                        ps = pspool.tile([P, BB], f32)
                        for dt_ in range(nt + 1):
                            j = PAIR_IDX[(nt, dt_)]
                            nc.tensor.matmul(
                                ps,
                                mm(w_sb[:, j * P : (j + 1) * P]),
                                mm(x_sb[:, dt_, :]),
                                start=(dt_ == 0),
                                stop=(dt_ == nt),
                            )
                        nc.vector.tensor_add(
                            out=o_sb[:, nt, :],
                            in0=ps,
                            in1=bias_sb[:, nt : nt + 1].broadcast_to([P, BB]),
                        )
                    nc.sync.dma_start(out=ov[:, :, bg * BB : (bg + 1) * BB], in_=o_sb)

            if reps == 1:
                body()
            else:
                with tc.For_i(0, reps, 1):
                    body()
    nc.compile()
    return nc


def _prep_shared(W, b, x_np_dt):
    # masked transposed weights, packed as the 21 lower-triangular 128x128 tiles
    Wm = W * np.tril(np.ones((D, D), np.float32), k=-1)
    WT = np.zeros((DP, DP), np.float32)
    WT[:D, :D] = Wm.T  # WT[d, n] = Wm[n, d]
    w_packed = np.empty((P, NPAIR, P), x_np_dt)
    for j, (nt, dt_) in enumerate(PAIRS):
        w_packed[:, j, :] = WT[dt_ * P : (dt_ + 1) * P, nt * P : (nt + 1) * P]
    w_packed = np.ascontiguousarray(w_packed.reshape(P, NPAIR * P))
    bias_pad = np.zeros(DP, np.float32)
    bias_pad[:D] = b
    bias_t = np.ascontiguousarray(bias_pad.reshape(NT, P).T)  # [p, t] = b[t*128+p]
    return w_packed, bias_t


def kernel(x, W, b):
    from concourse.bass_utils import run_bass_kernel_spmd

    x_np_dt = _np_dt(X_DT)
    out_np_dt = _np_dt(OUT_DT)
    nc = _build(BPC)
    w_packed, bias_t = _prep_shared(W, b, x_np_dt)

    in_maps = []
    for c in range(NCORES):
        xs = x[c * BPC : (c + 1) * BPC]
        xT = np.zeros((DP, BPC), x_np_dt)
        xT[:D] = xs.T
        in_maps.append({"xt": xT, "wt": w_packed, "bias": bias_t})

    res = run_bass_kernel_spmd(nc, in_maps, core_ids=list(range(NCORES)))

    out = np.empty((B, D), np.float32)
    for c in range(NCORES):
        out[c * BPC : (c + 1) * BPC] = (
            res.results[c]["outt"][:D].astype(np.float32).T
        )
    return out


# revision 8
# speedup vs baseline: 2.8330x; 2.8330x over previous
# FVSBN kernel for Trainium2: out = x @ (W * tril(-1)).T + b
#   x: [65536, 764] f32, W: [764, 764] f32, b: [764] f32 -> out: [65536, 764] f32
#
# Strategy: data-parallel over batch across 8 NeuronCores (8192 rows each).
# On each core we compute out^T = Wm^T-tiles.T @ x^T as a block-lower-triangular
# matmul: the strictly-lower-triangular mask means output tile row nt only needs
# contraction tiles dt <= nt (21 of 36 tile pairs).
#   - stationary operand (lhsT): Wm^T tile [128 d, 128 n]  (host packs the 21
#     used tiles contiguously)
#   - moving operand (rhs): x^T tile [128 d, 512 b]  (host pre-transposes x so
#     device DMAs are contiguous)
#   - psum [128 n, 512 b] accumulates over dt; eviction fuses the bias add
#     (per-partition scalar) on the vector engine.
# Host gathers by transposing each core's out^T back.

import numpy as np

B = 65536
D = 764
NCORES = 8
BPC = B // NCORES  # 8192 rows per core
P = 128
NT = 6  # ceil(764/128)
DP = NT * P  # 768, zero-padded depth
BB = 512  # matmul moving free dim == psum bank width (fp32)
PAIRS = [(nt, dt) for nt in range(NT) for dt in range(nt + 1)]
PAIR_IDX = {p: j for j, p in enumerate(PAIRS)}
NPAIR = len(PAIRS)  # 21

# device compute dtypes (np side); "float32r" = full-rate fp32 matmul mode
X_DT = "float16"  # dtype of x / W on device
OUT_DT = "float16"  # dtype out^T is written in
MM_DT = "float16"  # dtype the PE sees for the matmul operands


def _np_dt(name):
    import ml_dtypes

    return {
        "float32": np.float32,
        "float16": np.float16,
        "bfloat16": ml_dtypes.bfloat16,
    }[name]


def _build(bpc, x_dt_str=X_DT, out_dt_str=OUT_DT, mm_dt_str=MM_DT, reps=1):
    import concourse.mybir as mybir
    from concourse import bacc
    from concourse.tile import TileContext

    x_dt = getattr(mybir.dt, x_dt_str)
    out_dt = getattr(mybir.dt, out_dt_str)
    mm_dt = getattr(mybir.dt, mm_dt_str)
    f32 = mybir.dt.float32
    nbb = bpc // BB

    nc = bacc.Bacc("TRN2", target_bir_lowering=False, debug=False)
    xT = nc.dram_tensor("xt", [DP, bpc], x_dt, kind="ExternalInput")
    wt = nc.dram_tensor("wt", [P, NPAIR * P], x_dt, kind="ExternalInput")
    bias = nc.dram_tensor("bias", [P, NT], f32, kind="ExternalInput")
    outT = nc.dram_tensor("outt", [DP, bpc], out_dt, kind="ExternalOutput")

    def mm(ap):
        return ap if ap.dtype == mm_dt else ap.bitcast(mm_dt)

    # b is split in halves for x-load/compute pipelining; within a half,
    # 512-wide matmul blocks. x^T stays fully resident in SBUF (96KB/part fp16).
    HB = bpc // 2
    nhb = HB // BB

    with TileContext(nc) as tc:
        with (
            tc.tile_pool(name="wpool", bufs=1) as wpool,
            tc.tile_pool(name="bpool", bufs=1) as bpool,
            tc.tile_pool(name="xpool", bufs=1) as xpool,
            tc.tile_pool(name="opool", bufs=3) as opool,
            tc.tile_pool(name="pspool", bufs=8, space="PSUM") as pspool,
        ):
            w_sb = wpool.tile([P, NPAIR * P], x_dt)
            nc.sync.dma_start(out=w_sb, in_=wt.ap())
            bias_sb = bpool.tile([P, NT], f32)
            nc.sync.dma_start(out=bias_sb, in_=bias.ap())

            xres = [
                xpool.tile([P, bpc], x_dt, tag=f"xres{t}", name=f"xres{t}")
                for t in range(NT)
            ]

            def body():
                for half in range(2):
                    for t in range(NT):
                        nc.sync.dma_start(
                            out=xres[t][:, half * HB : (half + 1) * HB],
                            in_=xT.ap()[t * P : (t + 1) * P, half * HB : (half + 1) * HB],
                        )
                for half in range(2):
                    for nt in range(NT):
                        o_nt = opool.tile([P, HB], out_dt)
                        for bg in range(nhb):
                            c0 = half * HB + bg * BB
                            ps = pspool.tile([P, BB], f32)
                            for dt_ in range(nt + 1):
                                j = PAIR_IDX[(nt, dt_)]
                                nc.tensor.matmul(
                                    ps,
                                    mm(w_sb[:, j * P : (j + 1) * P]),
                                    mm(xres[dt_][:, c0 : c0 + BB]),
                                    start=(dt_ == 0),
                                    stop=(dt_ == nt),
                                )
                            nc.vector.tensor_add(
                                out=o_nt[:, bg * BB : (bg + 1) * BB],
                                in0=ps,
                                in1=bias_sb[:, nt : nt + 1].broadcast_to([P, BB]),
                            )
                        nc.sync.dma_start(
                            out=outT.ap()[
                                nt * P : (nt + 1) * P, half * HB : (half + 1) * HB
                            ],
                            in_=o_nt,
                        )

            if reps == 1:
                body()
            else:
                with tc.For_i(0, reps, 1):
                    body()
    nc.compile()
    return nc


def _prep_shared(W, b, x_np_dt):
    # masked transposed weights, packed as the 21 lower-triangular 128x128 tiles
    Wm = W * np.tril(np.ones((D, D), np.float32), k=-1)
    WT = np.zeros((DP, DP), np.float32)
    WT[:D, :D] = Wm.T  # WT[d, n] = Wm[n, d]
    w_packed = np.empty((P, NPAIR, P), x_np_dt)
    for j, (nt, dt_) in enumerate(PAIRS):
        w_packed[:, j, :] = WT[dt_ * P : (dt_ + 1) * P, nt * P : (nt + 1) * P]
    w_packed = np.ascontiguousarray(w_packed.reshape(P, NPAIR * P))
    bias_pad = np.zeros(DP, np.float32)
    bias_pad[:D] = b
    bias_t = np.ascontiguousarray(bias_pad.reshape(NT, P).T)  # [p, t] = b[t*128+p]
    return w_packed, bias_t


def kernel(x, W, b):
    from concourse.bass_utils import run_bass_kernel_spmd

    x_np_dt = _np_dt(X_DT)
    out_np_dt = _np_dt(OUT_DT)
    nc = _build(BPC)
    w_packed, bias_t = _prep_shared(W, b, x_np_dt)

    in_maps = []
    for c in range(NCORES):
        xs = x[c * BPC : (c + 1) * BPC]
        xT = np.zeros((DP, BPC), x_np_dt)
        xT[:D] = xs.T
        in_maps.append({"xt": xT, "wt": w_packed, "bias": bias_t})

    res = run_bass_kernel_spmd(nc, in_maps, core_ids=list(range(NCORES)))

    out = np.empty((B, D), np.float32)
    for c in range(NCORES):
        out[c * BPC : (c + 1) * BPC] = (
            res.results[c]["outt"][:D].astype(np.float32).T
        )
    return out
